# revision 15
# baseline (speedup 1.0000x reference)
"""Candidate-aware clock attention kernel for Trainium2 (Bass/Tile).

Computes, for each batch row b with history items items_pad[b, :L]:
    k    = item_emb[items_pad]            [B, L, D]
    q    = item_emb[cand_items]           [B, D]
    sim  = einsum('bld,bd->bl', k, q)
    gate = dt_gate[dts_pad][..., 0]       (row 0 of dt_gate zeroed)
    tau  = softplus(raw_tau) + 1e-6
    attn = softmax(where(mask, sim*gate/tau, -inf), axis=1)
    u    = einsum('bl,bld->bd', attn, k)
returns (u, attn, tau).

Sharding: data-parallel over the batch dim across 8 NeuronCores (512 rows
per core); the item_emb table is replicated. The embedding gather runs on
device via indirect DMA: one call per history column, gathering one 256B
row per partition (the only per-token gather primitive supported by this
runtime — the per-index dma_gather ucode is unavailable here). The tiny
128-entry dt_gate lookup is folded into host-side input prep together
with the 1/tau scale; all floating-point model compute (dot products,
gating, masked softmax, weighted sum) runs on device.
"""

import numpy as np

# Problem shapes (hardcoded per contract).
B, L, D, V = 4096, 200, 64, 100000
NBUCKETS = 128
NCORES = 8
BS = B // NCORES        # batch rows per core
P = 128                 # SBUF partitions
NT = BS // P            # row-tiles per core
LC = L + 1              # history columns + candidate column
NEG_BIG = -3.0e38       # additive mask value (finite, exp() underflows to 0)

_cache = {}


def _build_nc():
    """Build + compile the per-core Bass program (identical on all cores)."""
    import concourse.bacc as bacc
    import concourse.tile as tile
    from concourse import bass, mybir

    f32 = mybir.dt.float32
    i32 = mybir.dt.int32
    Alu = mybir.AluOpType
    Axis = mybir.AxisListType
    Act = mybir.ActivationFunctionType

    nc = bacc.Bacc(
        "TRN2", target_bir_lowering=False, debug=False,
        dynamic_dma_scratch_size=32768,
    )

    # itemsc = items_pad with cand_items appended as column L.
    itemsc = nc.dram_tensor("itemsc", [BS, LC], i32, kind="ExternalInput").ap()
    gate_in = nc.dram_tensor("gate", [BS, L], f32, kind="ExternalInput").ap()
    maskadd = nc.dram_tensor("maskadd", [BS, L], f32, kind="ExternalInput").ap()
    emb = nc.dram_tensor("emb", [V, D], f32, kind="ExternalInput").ap()
    u_out = nc.dram_tensor("u", [BS, D], f32, kind="ExternalOutput").ap()
    attn_out = nc.dram_tensor("attn", [BS, L], f32, kind="ExternalOutput").ap()

    with tile.TileContext(nc) as tc:
        with (
            tc.tile_pool(name="big", bufs=2) as big,
            tc.tile_pool(name="tmp", bufs=1) as tmp,
            tc.tile_pool(name="small", bufs=2) as small,
        ):
            for t in range(NT):
                r = slice(t * P, (t + 1) * P)

                idx = small.tile([P, LC], i32, tag="idx")
                nc.sync.dma_start(out=idx[:], in_=itemsc[r, :])
                gate = small.tile([P, L], f32, tag="gate")
                nc.sync.dma_start(out=gate[:], in_=gate_in[r, :])
                mka = small.tile([P, L], f32, tag="mka")
                nc.sync.dma_start(out=mka[:], in_=maskadd[r, :])

                # Gather k and q rows: per column, one indirect DMA moving one
                # 256B table row into each partition (row p <- emb[idx[p, c]]).
                kq = big.tile([P, LC * D], f32, tag="kq")
                for c in range(LC):
                    nc.gpsimd.indirect_dma_start(
                        out=kq[:, c * D:(c + 1) * D],
                        out_offset=None,
                        in_=emb,
                        in_offset=bass.IndirectOffsetOnAxis(
                            ap=idx[:, c:c + 1], axis=0
                        ),
                    )
                k = kq[:, : L * D]
                q = kq[:, L * D:]

                kv = k.rearrange("p (l d) -> p l d", d=D)
                prod = tmp.tile([P, L * D], f32, tag="prod")
                prodv = prod[:].rearrange("p (l d) -> p l d", d=D)
                qb = q.unsqueeze(1).broadcast_to([P, L, D])
                nc.vector.tensor_tensor(out=prodv, in0=kv, in1=qb, op=Alu.mult)
                sim = small.tile([P, L], f32, tag="sim")
                nc.vector.tensor_reduce(
                    out=sim[:], in_=prodv, axis=Axis.X, op=Alu.add
                )

                logits = small.tile([P, L], f32, tag="logits")
                nc.vector.tensor_tensor(
                    out=logits[:], in0=sim[:], in1=gate[:], op=Alu.mult
                )
                logits2 = small.tile([P, L], f32, tag="logits2")
                nc.vector.tensor_tensor(
                    out=logits2[:], in0=logits[:], in1=mka[:], op=Alu.add
                )

                negmx = small.tile([P, 1], f32, tag="negmx")
                nc.vector.tensor_reduce(
                    out=negmx[:], in_=logits2[:], axis=Axis.X, op=Alu.max,
                    negate=True,
                )
                e = small.tile([P, L], f32, tag="e")
                s = small.tile([P, 1], f32, tag="s")
                nc.scalar.activation(
                    out=e[:], in_=logits2[:], func=Act.Exp,
                    bias=negmx[:], scale=1.0, accum_out=s[:],
                )
                rs = small.tile([P, 1], f32, tag="rs")
                nc.vector.reciprocal(out=rs[:], in_=s[:])
                attn = small.tile([P, L], f32, tag="attn")
                nc.vector.tensor_scalar_mul(out=attn[:], in0=e[:], scalar1=rs[:])
                nc.sync.dma_start(out=attn_out[r, :], in_=attn[:])

                # u = sum_l attn[l] * k[l, :]  (reuse prod as the product buffer)
                ab = attn[:].unsqueeze(2).broadcast_to([P, L, D])
                nc.vector.tensor_tensor(out=prodv, in0=kv, in1=ab, op=Alu.mult)
                u = small.tile([P, D], f32, tag="u")
                nc.vector.tensor_reduce(
                    out=u[:],
                    in_=prod[:].rearrange("p (l d) -> p d l", d=D),
                    axis=Axis.X,
                    op=Alu.add,
                )
                nc.sync.dma_start(out=u_out[r, :], in_=u[:])

    nc.compile()
    return nc


def _get_nc():
    if "nc" not in _cache:
        _cache["nc"] = _build_nc()
    return _cache["nc"]


def kernel(items_pad, dts_pad, mask, cand_items, item_emb, dt_gate, raw_tau):
    from concourse.bass_utils import run_bass_kernel_spmd

    items_pad = np.ascontiguousarray(np.asarray(items_pad, dtype=np.int32))
    dts_pad = np.asarray(dts_pad, dtype=np.int64)
    cand_items = np.ascontiguousarray(np.asarray(cand_items, dtype=np.int32))
    mask = np.asarray(mask, dtype=bool)
    item_emb = np.ascontiguousarray(np.asarray(item_emb, dtype=np.float32))
    dt_gate = np.asarray(dt_gate, dtype=np.float32)

    # tau = softplus(raw_tau) + 1e-6, in f32 like the reference.
    rt = np.float32(np.asarray(raw_tau, dtype=np.float32))
    tau = np.float32(np.log1p(np.exp(rt))) + np.float32(1e-6)

    # dt_gate lookup with padding row zeroed, pre-divided by tau.
    gtab = dt_gate.reshape(NBUCKETS).copy()
    gtab[0] = 0.0
    gtab = (gtab / tau).astype(np.float32)
    gate = np.ascontiguousarray(gtab[dts_pad])               # [B, L] f32

    # Additive mask: 0 where valid, -3e38 where masked (exp underflows to 0).
    maskadd = np.where(mask, np.float32(0.0), np.float32(NEG_BIG)).astype(np.float32)

    itemsc = np.ascontiguousarray(
        np.concatenate([items_pad, cand_items.reshape(B, 1)], axis=1)
    )

    nc = _get_nc()
    in_maps = []
    for c in range(NCORES):
        r = slice(c * BS, (c + 1) * BS)
        in_maps.append(
            {
                "itemsc": itemsc[r],
                "gate": np.ascontiguousarray(gate[r]),
                "maskadd": np.ascontiguousarray(maskadd[r]),
                "emb": item_emb,
            }
        )

    res = run_bass_kernel_spmd(nc, in_maps, core_ids=list(range(NCORES)))
    _cache["last_results"] = res

    u = np.concatenate([res.results[c]["u"] for c in range(NCORES)], axis=0)
    attn = np.concatenate([res.results[c]["attn"] for c in range(NCORES)], axis=0)
    return u, attn, tau


# revision 16
# speedup vs baseline: 1.5549x; 1.5549x over previous
"""Candidate-aware clock attention kernel for Trainium2 (Bass/Tile).

Computes, for each batch row b with history items items_pad[b, :L]:
    k    = item_emb[items_pad]            [B, L, D]
    q    = item_emb[cand_items]           [B, D]
    sim  = einsum('bld,bd->bl', k, q)
    gate = dt_gate[dts_pad][..., 0]       (row 0 of dt_gate zeroed)
    tau  = softplus(raw_tau) + 1e-6
    attn = softmax(where(mask, sim*gate/tau, -inf), axis=1)
    u    = einsum('bl,bld->bd', attn, k)
returns (u, attn, tau).

Sharding: data-parallel over the batch dim across 8 NeuronCores (512 rows
per core); the item_emb table is replicated. The embedding gather runs on
device via indirect DMA, one call per history column gathering one 256B
table row into each partition (the only per-token gather primitive this
runtime supports — the per-index dma_gather ucode is unavailable here).
Since masked positions have exactly zero attention, each row's valid
tokens are left-packed on host (an index permutation) so the device only
gathers/computes max-valid-count columns; attn is scattered back to the
original positions on host (masked slots are exact zeros). The 128-entry
dt_gate lookup and the 1/tau scale are folded into host input prep; all
model floating-point compute (dot products, gating, masked softmax,
weighted sum) runs on device.
"""

import numpy as np

# Problem shapes (hardcoded per contract).
B, L, D, V = 4096, 200, 64, 100000
NBUCKETS = 128
NCORES = 8
BS = B // NCORES        # batch rows per core
P = 128                 # SBUF partitions
NT = BS // P            # row-tiles per core
NEG_BIG = -3.0e38       # additive mask value (finite, exp() underflows to 0)

_cache = {}


def _build_nc(cm):
    """Build + compile the per-core Bass program for cm history columns."""
    import concourse.bacc as bacc
    import concourse.tile as tile
    from concourse import bass, mybir

    f32 = mybir.dt.float32
    i32 = mybir.dt.int32
    Alu = mybir.AluOpType
    Axis = mybir.AxisListType
    Act = mybir.ActivationFunctionType

    cc = cm + 1  # + candidate column

    nc = bacc.Bacc("TRN2", target_bir_lowering=False, debug=False)

    # itemsc = left-packed valid items with cand_items appended as column cm.
    itemsc = nc.dram_tensor("itemsc", [BS, cc], i32, kind="ExternalInput").ap()
    gate_in = nc.dram_tensor("gate", [BS, cm], f32, kind="ExternalInput").ap()
    maskadd = nc.dram_tensor("maskadd", [BS, cm], f32, kind="ExternalInput").ap()
    emb = nc.dram_tensor("emb", [V, D], f32, kind="ExternalInput").ap()
    u_out = nc.dram_tensor("u", [BS, D], f32, kind="ExternalOutput").ap()
    attn_out = nc.dram_tensor("attn", [BS, cm], f32, kind="ExternalOutput").ap()

    with tile.TileContext(nc) as tc:
        with (
            tc.tile_pool(name="big", bufs=2) as big,
            tc.tile_pool(name="tmp", bufs=2) as tmp,
            tc.tile_pool(name="small", bufs=2) as small,
        ):
            for t in range(NT):
                r = slice(t * P, (t + 1) * P)

                idx = small.tile([P, cc], i32, tag="idx")
                nc.sync.dma_start(out=idx[:], in_=itemsc[r, :])
                gate = small.tile([P, cm], f32, tag="gate")
                nc.sync.dma_start(out=gate[:], in_=gate_in[r, :])
                mka = small.tile([P, cm], f32, tag="mka")
                nc.sync.dma_start(out=mka[:], in_=maskadd[r, :])

                # Gather k and q rows: per column, one indirect DMA moving one
                # 256B table row into each partition (row p <- emb[idx[p, c]]).
                kq = big.tile([P, cc * D], f32, tag="kq")
                for c in range(cc):
                    nc.gpsimd.indirect_dma_start(
                        out=kq[:, c * D:(c + 1) * D],
                        out_offset=None,
                        in_=emb,
                        in_offset=bass.IndirectOffsetOnAxis(
                            ap=idx[:, c:c + 1], axis=0
                        ),
                    )
                k = kq[:, : cm * D]
                q = kq[:, cm * D:]

                kv = k.rearrange("p (l d) -> p l d", d=D)
                prod = tmp.tile([P, cm * D], f32, tag="prod")
                prodv = prod[:].rearrange("p (l d) -> p l d", d=D)
                qb = q.unsqueeze(1).broadcast_to([P, cm, D])
                nc.vector.tensor_tensor(out=prodv, in0=kv, in1=qb, op=Alu.mult)
                sim = small.tile([P, cm], f32, tag="sim")
                nc.vector.tensor_reduce(
                    out=sim[:], in_=prodv, axis=Axis.X, op=Alu.add
                )

                logits = small.tile([P, cm], f32, tag="logits")
                nc.vector.tensor_tensor(
                    out=logits[:], in0=sim[:], in1=gate[:], op=Alu.mult
                )
                logits2 = small.tile([P, cm], f32, tag="logits2")
                nc.vector.tensor_tensor(
                    out=logits2[:], in0=logits[:], in1=mka[:], op=Alu.add
                )

                negmx = small.tile([P, 1], f32, tag="negmx")
                nc.vector.tensor_reduce(
                    out=negmx[:], in_=logits2[:], axis=Axis.X, op=Alu.max,
                    negate=True,
                )
                e = small.tile([P, cm], f32, tag="e")
                s = small.tile([P, 1], f32, tag="s")
                nc.scalar.activation(
                    out=e[:], in_=logits2[:], func=Act.Exp,
                    bias=negmx[:], scale=1.0, accum_out=s[:],
                )
                rs = small.tile([P, 1], f32, tag="rs")
                nc.vector.reciprocal(out=rs[:], in_=s[:])
                attn = small.tile([P, cm], f32, tag="attn")
                nc.vector.tensor_scalar_mul(out=attn[:], in0=e[:], scalar1=rs[:])
                nc.sync.dma_start(out=attn_out[r, :], in_=attn[:])

                # u = sum_l attn[l] * k[l, :]  (reuse prod as the product buffer)
                ab = attn[:].unsqueeze(2).broadcast_to([P, cm, D])
                nc.vector.tensor_tensor(out=prodv, in0=kv, in1=ab, op=Alu.mult)
                u = small.tile([P, D], f32, tag="u")
                nc.vector.tensor_reduce(
                    out=u[:],
                    in_=prod[:].rearrange("p (l d) -> p d l", d=D),
                    axis=Axis.X,
                    op=Alu.add,
                )
                nc.sync.dma_start(out=u_out[r, :], in_=u[:])

    nc.compile()
    return nc


def _get_nc(cm):
    key = ("nc", cm)
    if key not in _cache:
        _cache[key] = _build_nc(cm)
    return _cache[key]


def kernel(items_pad, dts_pad, mask, cand_items, item_emb, dt_gate, raw_tau):
    from concourse.bass_utils import run_bass_kernel_spmd

    items_pad = np.asarray(items_pad, dtype=np.int32)
    dts_pad = np.asarray(dts_pad, dtype=np.int64)
    cand_items = np.ascontiguousarray(np.asarray(cand_items, dtype=np.int32))
    mask = np.asarray(mask, dtype=bool)
    item_emb = np.ascontiguousarray(np.asarray(item_emb, dtype=np.float32))
    dt_gate = np.asarray(dt_gate, dtype=np.float32)

    # tau = softplus(raw_tau) + 1e-6, in f32 like the reference.
    rt = np.float32(np.asarray(raw_tau, dtype=np.float32))
    tau = np.float32(np.log1p(np.exp(rt))) + np.float32(1e-6)

    # dt_gate lookup with padding row zeroed, pre-divided by tau.
    gtab = dt_gate.reshape(NBUCKETS).copy()
    gtab[0] = 0.0
    gtab = (gtab / tau).astype(np.float32)
    gate_full = gtab[dts_pad].astype(np.float32)             # [B, L]

    # Left-pack each row's valid positions (stable: ascending l), pad after.
    order = np.argsort(~mask, axis=1, kind="stable")         # [B, L] positions
    nv = mask.sum(axis=1)
    cm = int(nv.max())                                       # history columns
    pos = np.ascontiguousarray(order[:, :cm])                # [B, cm]
    mask_c = np.take_along_axis(mask, pos, axis=1)           # [B, cm] packed mask
    items_c = np.take_along_axis(items_pad, pos, axis=1)
    gate_c = np.take_along_axis(gate_full, pos, axis=1)
    maskadd_c = np.where(mask_c, np.float32(0.0), np.float32(NEG_BIG)).astype(
        np.float32
    )
    itemsc = np.ascontiguousarray(
        np.concatenate([items_c, cand_items.reshape(B, 1)], axis=1)
    )
    gate_c = np.ascontiguousarray(gate_c)

    nc = _get_nc(cm)
    in_maps = []
    for c in range(NCORES):
        r = slice(c * BS, (c + 1) * BS)
        in_maps.append(
            {
                "itemsc": itemsc[r],
                "gate": gate_c[r],
                "maskadd": np.ascontiguousarray(maskadd_c[r]),
                "emb": item_emb,
            }
        )

    res = run_bass_kernel_spmd(nc, in_maps, core_ids=list(range(NCORES)))
    _cache["last_results"] = res

    u = np.concatenate([res.results[c]["u"] for c in range(NCORES)], axis=0)
    attn_c = np.concatenate(
        [res.results[c]["attn"] for c in range(NCORES)], axis=0
    )
    # Scatter packed attn back to original positions; masked slots are exact 0
    # (their packed value is exp(-3e38 - m)/Z = 0 or they were never packed).
    attn = np.zeros((B, L), dtype=np.float32)
    np.put_along_axis(attn, pos, attn_c * mask_c, axis=1)
    return u, attn, tau
